# revision 14
# baseline (speedup 1.0000x reference)
import os
import sys
sys.path.insert(0, '/opt/trn_rl_repo')
import numpy as np

try:
    import concourse.bass as bass
    from concourse import bacc, tile, mybir
    from concourse.bass_utils import run_bass_kernel_spmd
    _HAVE_BASS = True
except Exception:
    _HAVE_BASS = False

# ---- problem constants (hardcoded; kernel.py must be self-contained) ----
B, C, H, W = 4, 1, 1024, 1024
L = 16
UP = 2
N_CORES = 8
NPASS = 16                      # 4 ktypes x 4 rotations
ROWS = H // 2                   # shard: (batch, half) -> 8 shards of 512 rows
PIX = ROWS * W                  # 524288 pixels per core

# flat indirect-DMA gather geometry (one instruction per SBUF partition line)
# R/16 (the SWDGE packet row count) must be a multiple of 32 so the
# packet-boundary garbage rows coincide with the 32-offset burst garbage rows
R = 3584                        # rows per line: 28*128 (desc ring) = 112*32
UROW = (np.arange(R) % 32) != 0  # first row of each 32-burst/packet is garbage
U = int(UROW.sum())             # 3472 useful pixels per line
NGRP = 8                        # pixel groups per generation (16 passes each)
GPIX = NGRP * U                 # 30752 pixels per generation
NGEN = -(-PIX // GPIX)          # 19 generations
NSUB = R * 4 // 512             # 31 matmul sub-chunks per generation

OFFSETS = {
    'h': ((0, 0), (0, 1), (0, 2), (0, 3)),
    'd': ((0, 0), (1, 1), (2, 2), (3, 3)),
    't': ((0, 0), (2, 1), (3, 1), (3, 2)),
    'b': ((0, 0), (1, 2), (1, 3), (2, 3)),
}
KTYPES = ('h', 'd', 't', 'b')

_nc_cache = {}
LAST_STATS = {}


def _build_nc():
    """Flat-gather kernel: per generation, 128 indirect-DMA gathers (one per
    partition line; line = pixel-group x pass) from a concatenated 16-pass
    table; PE mask-matmul sums the 16 pass-partitions of each group into
    PSUM; ACT copies to SBUF; HWDGE writes out."""
    if 'nc' in _nc_cache:
        return _nc_cache['nc']
    f32 = mybir.dt.float32
    nc = bacc.Bacc('TRN2', target_bir_lowering=False,
                   dynamic_dma_scratch_size=2 ** 16)
    tab_d = nc.dram_tensor('tab', [NPASS * L ** 4, 4], f32, kind='ExternalInput')
    msk_d = nc.dram_tensor('msk', [128, NGRP], f32, kind='ExternalInput')
    idx_d = nc.dram_tensor('idx', [NGEN, 128, R], mybir.dt.int32,
                           kind='ExternalInput')
    out_d = nc.dram_tensor('out', [NGEN, NSUB, NGRP, 512], f32,
                           kind='ExternalOutput')

    with tile.TileContext(nc) as tc:
        with tc.tile_pool(name='const', bufs=1) as cpool, \
             tc.tile_pool(name='idx', bufs=2) as ipool, \
             tc.tile_pool(name='acc', bufs=2) as apool, \
             tc.tile_pool(name='oc', bufs=4) as opool, \
             tc.tile_pool(name='psum', bufs=8, space='PSUM') as ppool:
            msk = cpool.tile([128, NGRP], f32, tag='msk')
            nc.sync.dma_start(msk[:], msk_d[:])
            for gen in range(NGEN):
                it = ipool.tile([128, R], mybir.dt.int32, tag='it')
                nc.sync.dma_start(it[:], idx_d[gen])
                acc = apool.tile([128, R, 4], f32, tag='acc')
                for q in range(128):
                    nc.gpsimd.indirect_dma_start(
                        out=acc[q:q + 1, :, :], out_offset=None, in_=tab_d[:],
                        in_offset=bass.IndirectOffsetOnAxis(
                            ap=it[:, q * (R // 128):(q + 1) * (R // 128)], axis=0),
                        compute_op=mybir.AluOpType.bypass)
                for s in range(NSUB):
                    ps = ppool.tile([NGRP, 512], f32, tag='ps')
                    nc.tensor.matmul(ps[:], msk[:],
                                     acc[:, s * 128:(s + 1) * 128, :],
                                     start=True, stop=True)
                    oc = opool.tile([NGRP, 512], f32, tag='oc')
                    nc.scalar.copy(oc[:], ps[:])
                    nc.sync.dma_start(out_d[gen, s], oc[:])
    nc.compile()
    _nc_cache['nc'] = nc
    return nc


def _rot_offsets(offs, r):
    out = []
    for (dy, dx) in offs:
        if r == 0:
            out.append((dy, dx))
        elif r == 1:
            out.append((dx, -dy))
        elif r == 2:
            out.append((-dy, -dx))
        else:
            out.append((-dx, dy))
    return out


def _chan_perm(r):
    """perm[p*2+q] = source channel (i*2+j) that lands on subpixel (p, q)."""
    perm = [0] * 4
    for i in range(2):
        for j in range(2):
            if r == 0:
                p, q = i, j
            elif r == 1:
                p, q = j, 1 - i
            elif r == 2:
                p, q = 1 - i, 1 - j
            else:
                p, q = 1 - j, i
            perm[p * 2 + q] = i * 2 + j
    return perm


def _pass_tables_and_indices(img, weights):
    pad = np.pad(img[:, 0].astype(np.uint16), ((0, 0), (3, 3), (3, 3)),
                 mode='reflect')
    tabs = np.empty((NPASS, L ** 4, 4), np.float32)
    idx_full = np.empty((NPASS, B, H, W), np.int32)
    pi = 0
    for kt in KTYPES:
        for r in range(4):
            perm = _chan_perm(r)
            tabs[pi] = weights[kt][:, perm].astype(np.float32) * 0.25
            taps = _rot_offsets(OFFSETS[kt], r)
            acc = np.zeros((B, H, W), np.uint16)
            for (dy, dx) in taps:
                acc = (acc << 4) + pad[:, 3 + dy:3 + dy + H, 3 + dx:3 + dx + W]
            idx_full[pi] = acc
            pi += 1
    return tabs, idx_full


# static line geometry, shared by packing and unpacking
_JMAP = np.where(UROW, np.cumsum(UROW) - 1, 0).astype(np.int64)   # [R]


def kernel(img_lr, h_weight, d_weight, t_weight, b_weight, L=16, upscale=2):
    img = np.asarray(img_lr)
    weights = {'h': np.asarray(h_weight), 'd': np.asarray(d_weight),
               't': np.asarray(t_weight), 'b': np.asarray(b_weight)}
    tabs, idx_full = _pass_tables_and_indices(img, weights)

    use_device = _HAVE_BASS and bool(int(os.environ.get('HDTBLUT_DEVICE', '1')))
    planars = None
    LAST_STATS.clear()
    if use_device:
        try:
            planars = _run_device(tabs, idx_full)
            LAST_STATS['device_ok'] = True
        except Exception:
            if int(os.environ.get('HDTBLUT_STRICT', '0')):
                raise
            LAST_STATS['device_ok'] = False
            planars = None
    if planars is None:
        planars = []
        for core in range(N_CORES):
            b_, half = core // 2, core % 2
            sl = idx_full[:, b_, half * ROWS:(half + 1) * ROWS]
            acc = np.zeros((PIX, 4), np.float32)
            for p in range(NPASS):
                acc += tabs[p][sl[p].reshape(-1)]
            planars.append(acc)

    out = np.empty((B, 1, H * UP, W * UP), np.float32)
    for core in range(N_CORES):
        b_, half = core // 2, core % 2
        planar = np.asarray(planars[core]).reshape(ROWS, W, 2, 2)
        blk = planar.transpose(0, 2, 1, 3).reshape(ROWS * 2, W * 2)
        out[b_, 0, half * ROWS * 2:(half + 1) * ROWS * 2] = blk
    return out


def _run_device(tabs, idx_full):
    nc = _build_nc()
    tab16 = tabs.reshape(NPASS * L ** 4, 4)
    msk = np.zeros((128, NGRP), np.float32)
    for q in range(128):
        msk[q, q // NPASS] = 1.0

    # static pixel map [NGEN, NGRP, R]
    base = (np.arange(NGEN)[:, None, None] * GPIX
            + np.arange(NGRP)[None, :, None] * U + _JMAP[None, None, :])
    base = np.minimum(base, PIX + GPIX - 1)

    in_maps = []
    for core in range(N_CORES):
        b_, half = core // 2, core % 2
        sl = idx_full[:, b_, half * ROWS:(half + 1) * ROWS]
        idx16 = (sl.reshape(NPASS, PIX)
                 + (np.arange(NPASS, dtype=np.int32) * L ** 4)[:, None])
        padded = np.concatenate(
            [idx16, np.zeros((NPASS, GPIX), np.int32)], axis=1)
        # offsets[gen, q=16g+p, j] = padded[p, base[gen, g, j]]
        offs = padded[:, base]                       # [NPASS, NGEN, NGRP, R]
        offs = offs.transpose(1, 2, 0, 3)            # [NGEN, NGRP, NPASS, R]
        offs = offs.reshape(NGEN, 128, R)            # line q = 16g+p
        # DGE consumption: o_j = it[j%128, j//128] within each line's 31-col view
        wr = offs.reshape(NGEN, 128, R // 128, 128).transpose(0, 3, 1, 2)
        wr = wr.reshape(NGEN, 128, R).astype(np.int32)
        in_maps.append({'tab': tab16, 'msk': msk, 'idx': np.ascontiguousarray(wr)})

    import time as _time
    t0 = _time.time()
    res = run_bass_kernel_spmd(nc, in_maps, core_ids=list(range(N_CORES)))
    LAST_STATS['exec_wall_s'] = round(_time.time() - t0, 3)
    if res.exec_time_ns:
        LAST_STATS['hw_exec_ns'] = res.exec_time_ns

    planars = []
    for core in range(N_CORES):
        od = np.asarray(res.results[core]['out'])    # [NGEN, NSUB, NGRP, 512]
        vals = od.transpose(0, 2, 1, 3).reshape(NGEN, NGRP, R, 4)
        planar = vals[:, :, UROW, :].reshape(NGEN * GPIX, 4)[:PIX]
        planars.append(planar)
    return planars


# revision 15
# speedup vs baseline: 1.0151x; 1.0151x over previous
import os
import sys
sys.path.insert(0, '/opt/trn_rl_repo')
import numpy as np

try:
    import concourse.bass as bass
    from concourse import bacc, tile, mybir
    from concourse.bass_utils import run_bass_kernel_spmd
    _HAVE_BASS = True
except Exception:
    _HAVE_BASS = False

# ---- problem constants (hardcoded; kernel.py must be self-contained) ----
B, C, H, W = 4, 1, 1024, 1024
L = 16
UP = 2
N_CORES = 8
NPASS = 16                      # 4 ktypes x 4 rotations
ROWS = H // 2                   # shard: (batch, half) -> 8 shards of 512 rows
PIX = ROWS * W                  # 524288 pixels per core

# flat indirect-DMA gather geometry (one instruction per SBUF partition line)
# first row of every 32-offset burst and of every SWDGE packet (R/16 rows)
# receives a corrupted offset -> throwaway slots, discarded at unpack
R = 3840                        # rows per line: 30*128; packet stride 240
UROW = ((np.arange(R) % 32) != 0) & ((np.arange(R) % (R // 16)) != 0)
U = int(UROW.sum())             # 3712 useful pixels per line
NGRP = 8                        # pixel groups per generation (16 passes each)
GPIX = NGRP * U                 # 30752 pixels per generation
NGEN = -(-PIX // GPIX)          # 19 generations
NSUB = R * 4 // 512             # 31 matmul sub-chunks per generation

OFFSETS = {
    'h': ((0, 0), (0, 1), (0, 2), (0, 3)),
    'd': ((0, 0), (1, 1), (2, 2), (3, 3)),
    't': ((0, 0), (2, 1), (3, 1), (3, 2)),
    'b': ((0, 0), (1, 2), (1, 3), (2, 3)),
}
KTYPES = ('h', 'd', 't', 'b')

_nc_cache = {}
LAST_STATS = {}


def _build_nc():
    """Flat-gather kernel: per generation, 128 indirect-DMA gathers (one per
    partition line; line = pixel-group x pass) from a concatenated 16-pass
    table; PE mask-matmul sums the 16 pass-partitions of each group into
    PSUM; ACT copies to SBUF; HWDGE writes out."""
    if 'nc' in _nc_cache:
        return _nc_cache['nc']
    f32 = mybir.dt.float32
    nc = bacc.Bacc('TRN2', target_bir_lowering=False,
                   dynamic_dma_scratch_size=2 ** 16)
    tab_d = nc.dram_tensor('tab', [NPASS * L ** 4, 4], f32, kind='ExternalInput')
    msk_d = nc.dram_tensor('msk', [128, NGRP], f32, kind='ExternalInput')
    idx_d = nc.dram_tensor('idx', [NGEN, 128, R], mybir.dt.int32,
                           kind='ExternalInput')
    out_d = nc.dram_tensor('out', [NGEN, NSUB, NGRP, 512], f32,
                           kind='ExternalOutput')

    with tile.TileContext(nc) as tc:
        with tc.tile_pool(name='const', bufs=1) as cpool, \
             tc.tile_pool(name='idx', bufs=2) as ipool, \
             tc.tile_pool(name='acc', bufs=2) as apool, \
             tc.tile_pool(name='oc', bufs=4) as opool, \
             tc.tile_pool(name='psum', bufs=8, space='PSUM') as ppool:
            msk = cpool.tile([128, NGRP], f32, tag='msk')
            nc.sync.dma_start(msk[:], msk_d[:])
            for gen in range(NGEN):
                it = ipool.tile([128, R], mybir.dt.int32, tag='it')
                nc.sync.dma_start(it[:], idx_d[gen])
                acc = apool.tile([128, R, 4], f32, tag='acc')
                for q in range(128):
                    nc.gpsimd.indirect_dma_start(
                        out=acc[q:q + 1, :, :], out_offset=None, in_=tab_d[:],
                        in_offset=bass.IndirectOffsetOnAxis(
                            ap=it[:, q * (R // 128):(q + 1) * (R // 128)], axis=0),
                        compute_op=mybir.AluOpType.bypass)
                for s in range(NSUB):
                    ps = ppool.tile([NGRP, 512], f32, tag='ps')
                    nc.tensor.matmul(ps[:], msk[:],
                                     acc[:, s * 128:(s + 1) * 128, :],
                                     start=True, stop=True)
                    oc = opool.tile([NGRP, 512], f32, tag='oc')
                    nc.scalar.copy(oc[:], ps[:])
                    nc.sync.dma_start(out_d[gen, s], oc[:])
    nc.compile()
    _nc_cache['nc'] = nc
    return nc


def _rot_offsets(offs, r):
    out = []
    for (dy, dx) in offs:
        if r == 0:
            out.append((dy, dx))
        elif r == 1:
            out.append((dx, -dy))
        elif r == 2:
            out.append((-dy, -dx))
        else:
            out.append((-dx, dy))
    return out


def _chan_perm(r):
    """perm[p*2+q] = source channel (i*2+j) that lands on subpixel (p, q)."""
    perm = [0] * 4
    for i in range(2):
        for j in range(2):
            if r == 0:
                p, q = i, j
            elif r == 1:
                p, q = j, 1 - i
            elif r == 2:
                p, q = 1 - i, 1 - j
            else:
                p, q = 1 - j, i
            perm[p * 2 + q] = i * 2 + j
    return perm


def _pass_tables_and_indices(img, weights):
    pad = np.pad(img[:, 0].astype(np.uint16), ((0, 0), (3, 3), (3, 3)),
                 mode='reflect')
    tabs = np.empty((NPASS, L ** 4, 4), np.float32)
    idx_full = np.empty((NPASS, B, H, W), np.int32)
    pi = 0
    for kt in KTYPES:
        for r in range(4):
            perm = _chan_perm(r)
            tabs[pi] = weights[kt][:, perm].astype(np.float32) * 0.25
            taps = _rot_offsets(OFFSETS[kt], r)
            acc = np.zeros((B, H, W), np.uint16)
            for (dy, dx) in taps:
                acc = (acc << 4) + pad[:, 3 + dy:3 + dy + H, 3 + dx:3 + dx + W]
            idx_full[pi] = acc
            pi += 1
    return tabs, idx_full


# static line geometry, shared by packing and unpacking
_JMAP = np.where(UROW, np.cumsum(UROW) - 1, 0).astype(np.int64)   # [R]


def kernel(img_lr, h_weight, d_weight, t_weight, b_weight, L=16, upscale=2):
    img = np.asarray(img_lr)
    weights = {'h': np.asarray(h_weight), 'd': np.asarray(d_weight),
               't': np.asarray(t_weight), 'b': np.asarray(b_weight)}
    tabs, idx_full = _pass_tables_and_indices(img, weights)

    use_device = _HAVE_BASS and bool(int(os.environ.get('HDTBLUT_DEVICE', '1')))
    planars = None
    LAST_STATS.clear()
    if use_device:
        try:
            planars = _run_device(tabs, idx_full)
            LAST_STATS['device_ok'] = True
        except Exception:
            if int(os.environ.get('HDTBLUT_STRICT', '0')):
                raise
            LAST_STATS['device_ok'] = False
            planars = None
    if planars is None:
        planars = []
        for core in range(N_CORES):
            b_, half = core // 2, core % 2
            sl = idx_full[:, b_, half * ROWS:(half + 1) * ROWS]
            acc = np.zeros((PIX, 4), np.float32)
            for p in range(NPASS):
                acc += tabs[p][sl[p].reshape(-1)]
            planars.append(acc)

    out = np.empty((B, 1, H * UP, W * UP), np.float32)
    for core in range(N_CORES):
        b_, half = core // 2, core % 2
        planar = np.asarray(planars[core]).reshape(ROWS, W, 2, 2)
        blk = planar.transpose(0, 2, 1, 3).reshape(ROWS * 2, W * 2)
        out[b_, 0, half * ROWS * 2:(half + 1) * ROWS * 2] = blk
    return out


def _run_device(tabs, idx_full):
    nc = _build_nc()
    tab16 = tabs.reshape(NPASS * L ** 4, 4)
    msk = np.zeros((128, NGRP), np.float32)
    for q in range(128):
        msk[q, q // NPASS] = 1.0

    # static pixel map [NGEN, NGRP, R]
    base = (np.arange(NGEN)[:, None, None] * GPIX
            + np.arange(NGRP)[None, :, None] * U + _JMAP[None, None, :])
    base = np.minimum(base, PIX + GPIX - 1)

    in_maps = []
    for core in range(N_CORES):
        b_, half = core // 2, core % 2
        sl = idx_full[:, b_, half * ROWS:(half + 1) * ROWS]
        idx16 = (sl.reshape(NPASS, PIX)
                 + (np.arange(NPASS, dtype=np.int32) * L ** 4)[:, None])
        padded = np.concatenate(
            [idx16, np.zeros((NPASS, GPIX), np.int32)], axis=1)
        # offsets[gen, q=16g+p, j] = padded[p, base[gen, g, j]]
        offs = padded[:, base]                       # [NPASS, NGEN, NGRP, R]
        offs = offs.transpose(1, 2, 0, 3)            # [NGEN, NGRP, NPASS, R]
        offs = offs.reshape(NGEN, 128, R)            # line q = 16g+p
        # DGE consumption: o_j = it[j%128, j//128] within each line's 31-col view
        wr = offs.reshape(NGEN, 128, R // 128, 128).transpose(0, 3, 1, 2)
        wr = wr.reshape(NGEN, 128, R).astype(np.int32)
        in_maps.append({'tab': tab16, 'msk': msk, 'idx': np.ascontiguousarray(wr)})

    import time as _time
    t0 = _time.time()
    res = run_bass_kernel_spmd(nc, in_maps, core_ids=list(range(N_CORES)))
    LAST_STATS['exec_wall_s'] = round(_time.time() - t0, 3)
    if res.exec_time_ns:
        LAST_STATS['hw_exec_ns'] = res.exec_time_ns

    planars = []
    for core in range(N_CORES):
        od = np.asarray(res.results[core]['out'])    # [NGEN, NSUB, NGRP, 512]
        vals = od.transpose(0, 2, 1, 3).reshape(NGEN, NGRP, R, 4)
        planar = vals[:, :, UROW, :].reshape(NGEN * GPIX, 4)[:PIX]
        planars.append(planar)
    return planars


# revision 17
# speedup vs baseline: 1.0292x; 1.0139x over previous
import os
import sys
sys.path.insert(0, '/opt/trn_rl_repo')
import numpy as np

try:
    import concourse.bass as bass
    from concourse import bacc, tile, mybir
    from concourse.bass_utils import run_bass_kernel_spmd
    _HAVE_BASS = True
except Exception:
    _HAVE_BASS = False

# ---- problem constants (hardcoded; kernel.py must be self-contained) ----
B, C, H, W = 4, 1, 1024, 1024
L = 16
UP = 2
N_CORES = 8
NPASS = 16                      # 4 ktypes x 4 rotations
ROWS = H // 2                   # shard: (batch, half) -> 8 shards of 512 rows
PIX = ROWS * W                  # 524288 pixels per core

# flat indirect-DMA gather geometry (one instruction per SBUF partition line)
# first row of every 32-offset burst and of every SWDGE packet (R/16 rows)
# receives a corrupted offset -> throwaway slots, discarded at unpack
R = 3840                        # rows per line: 30*128; packet stride 240
UROW = ((np.arange(R) % 32) != 0) & ((np.arange(R) % (R // 16)) != 0)
U = int(UROW.sum())             # 3712 useful pixels per line
NGRP = 8                        # pixel groups per generation (16 passes each)
GPIX = NGRP * U                 # 30752 pixels per generation
NGEN = -(-PIX // GPIX)          # 18 generations
NSUB = R * 4 // 512             # 31 matmul sub-chunks per generation

OFFSETS = {
    'h': ((0, 0), (0, 1), (0, 2), (0, 3)),
    'd': ((0, 0), (1, 1), (2, 2), (3, 3)),
    't': ((0, 0), (2, 1), (3, 1), (3, 2)),
    'b': ((0, 0), (1, 2), (1, 3), (2, 3)),
}
KTYPES = ('h', 'd', 't', 'b')

_nc_cache = {}
LAST_STATS = {}


def _build_nc():
    """Flat-gather kernel: per generation, 128 indirect-DMA gathers (one per
    partition line; line = pixel-group x pass) from a concatenated 16-pass
    table; PE mask-matmul sums the 16 pass-partitions of each group into
    PSUM; ACT copies to SBUF; HWDGE writes out."""
    if 'nc' in _nc_cache:
        return _nc_cache['nc']
    f32 = mybir.dt.float32
    nc = bacc.Bacc('TRN2', target_bir_lowering=False,
                   dynamic_dma_scratch_size=2 ** 16)
    tab_d = nc.dram_tensor('tab', [NPASS * L ** 4, 4], f32, kind='ExternalInput')
    msk_d = nc.dram_tensor('msk', [128, NGRP], f32, kind='ExternalInput')
    idx_d = nc.dram_tensor('idx', [NGEN, 128, R], mybir.dt.int32,
                           kind='ExternalInput')
    out_d = nc.dram_tensor('out', [NGEN, NSUB, NGRP, 512], f32,
                           kind='ExternalOutput')

    with tile.TileContext(nc) as tc:
        with tc.tile_pool(name='const', bufs=1) as cpool, \
             tc.tile_pool(name='idx', bufs=2) as ipool, \
             tc.tile_pool(name='acc', bufs=2) as apool, \
             tc.tile_pool(name='oc', bufs=4) as opool, \
             tc.tile_pool(name='psum', bufs=8, space='PSUM') as ppool:
            msk = cpool.tile([128, NGRP], f32, tag='msk')
            nc.sync.dma_start(msk[:], msk_d[:])
            for gen in range(NGEN):
                it = ipool.tile([128, R], mybir.dt.int32, tag='it')
                nc.sync.dma_start(it[:], idx_d[gen])
                acc = apool.tile([128, R, 4], f32, tag='acc')
                # last generation: only the line-groups that cover PIX; the
                # skipped partitions hold stale (finite) data from an earlier
                # generation and their output positions are discarded anyway
                ngrp_gen = -(-(PIX - gen * GPIX) // U) if gen == NGEN - 1 else NGRP
                for q in range(16 * ngrp_gen):
                    nc.gpsimd.indirect_dma_start(
                        out=acc[q:q + 1, :, :], out_offset=None, in_=tab_d[:],
                        in_offset=bass.IndirectOffsetOnAxis(
                            ap=it[:, q * (R // 128):(q + 1) * (R // 128)], axis=0),
                        compute_op=mybir.AluOpType.bypass)
                for s in range(NSUB):
                    ps = ppool.tile([NGRP, 512], f32, tag='ps')
                    nc.tensor.matmul(ps[:], msk[:],
                                     acc[:, s * 128:(s + 1) * 128, :],
                                     start=True, stop=True)
                    oc = opool.tile([NGRP, 512], f32, tag='oc')
                    nc.scalar.copy(oc[:], ps[:])
                    nc.sync.dma_start(out_d[gen, s], oc[:])
    nc.compile()
    _nc_cache['nc'] = nc
    return nc


def _rot_offsets(offs, r):
    out = []
    for (dy, dx) in offs:
        if r == 0:
            out.append((dy, dx))
        elif r == 1:
            out.append((dx, -dy))
        elif r == 2:
            out.append((-dy, -dx))
        else:
            out.append((-dx, dy))
    return out


def _chan_perm(r):
    """perm[p*2+q] = source channel (i*2+j) that lands on subpixel (p, q)."""
    perm = [0] * 4
    for i in range(2):
        for j in range(2):
            if r == 0:
                p, q = i, j
            elif r == 1:
                p, q = j, 1 - i
            elif r == 2:
                p, q = 1 - i, 1 - j
            else:
                p, q = 1 - j, i
            perm[p * 2 + q] = i * 2 + j
    return perm


def _pass_tables_and_indices(img, weights):
    pad = np.pad(img[:, 0].astype(np.uint16), ((0, 0), (3, 3), (3, 3)),
                 mode='reflect')
    tabs = np.empty((NPASS, L ** 4, 4), np.float32)
    idx_full = np.empty((NPASS, B, H, W), np.int32)
    pi = 0
    for kt in KTYPES:
        for r in range(4):
            perm = _chan_perm(r)
            tabs[pi] = weights[kt][:, perm].astype(np.float32) * 0.25
            taps = _rot_offsets(OFFSETS[kt], r)
            acc = np.zeros((B, H, W), np.uint16)
            for (dy, dx) in taps:
                acc = (acc << 4) + pad[:, 3 + dy:3 + dy + H, 3 + dx:3 + dx + W]
            idx_full[pi] = acc
            pi += 1
    return tabs, idx_full


# static line geometry, shared by packing and unpacking
_JMAP = np.where(UROW, np.cumsum(UROW) - 1, 0).astype(np.int64)   # [R]


def kernel(img_lr, h_weight, d_weight, t_weight, b_weight, L=16, upscale=2):
    img = np.asarray(img_lr)
    weights = {'h': np.asarray(h_weight), 'd': np.asarray(d_weight),
               't': np.asarray(t_weight), 'b': np.asarray(b_weight)}
    tabs, idx_full = _pass_tables_and_indices(img, weights)

    use_device = _HAVE_BASS and bool(int(os.environ.get('HDTBLUT_DEVICE', '1')))
    planars = None
    LAST_STATS.clear()
    if use_device:
        try:
            planars = _run_device(tabs, idx_full)
            LAST_STATS['device_ok'] = True
        except Exception:
            if int(os.environ.get('HDTBLUT_STRICT', '0')):
                raise
            LAST_STATS['device_ok'] = False
            planars = None
    if planars is None:
        planars = []
        for core in range(N_CORES):
            b_, half = core // 2, core % 2
            sl = idx_full[:, b_, half * ROWS:(half + 1) * ROWS]
            acc = np.zeros((PIX, 4), np.float32)
            for p in range(NPASS):
                acc += tabs[p][sl[p].reshape(-1)]
            planars.append(acc)

    out = np.empty((B, 1, H * UP, W * UP), np.float32)
    for core in range(N_CORES):
        b_, half = core // 2, core % 2
        planar = np.asarray(planars[core]).reshape(ROWS, W, 2, 2)
        blk = planar.transpose(0, 2, 1, 3).reshape(ROWS * 2, W * 2)
        out[b_, 0, half * ROWS * 2:(half + 1) * ROWS * 2] = blk
    return out


def _run_device(tabs, idx_full):
    nc = _build_nc()
    tab16 = tabs.reshape(NPASS * L ** 4, 4)
    msk = np.zeros((128, NGRP), np.float32)
    for q in range(128):
        msk[q, q // NPASS] = 1.0

    # static pixel map [NGEN, NGRP, R]
    base = (np.arange(NGEN)[:, None, None] * GPIX
            + np.arange(NGRP)[None, :, None] * U + _JMAP[None, None, :])
    base = np.minimum(base, PIX + GPIX - 1)

    in_maps = []
    for core in range(N_CORES):
        b_, half = core // 2, core % 2
        sl = idx_full[:, b_, half * ROWS:(half + 1) * ROWS]
        idx16 = (sl.reshape(NPASS, PIX)
                 + (np.arange(NPASS, dtype=np.int32) * L ** 4)[:, None])
        padded = np.concatenate(
            [idx16, np.zeros((NPASS, GPIX), np.int32)], axis=1)
        # offsets[gen, q=16g+p, j] = padded[p, base[gen, g, j]]
        offs = padded[:, base]                       # [NPASS, NGEN, NGRP, R]
        offs = offs.transpose(1, 2, 0, 3)            # [NGEN, NGRP, NPASS, R]
        offs = offs.reshape(NGEN, 128, R)            # line q = 16g+p
        # DGE consumption: o_j = it[j%128, j//128] within each line's 31-col view
        wr = offs.reshape(NGEN, 128, R // 128, 128).transpose(0, 3, 1, 2)
        wr = wr.reshape(NGEN, 128, R).astype(np.int32)
        in_maps.append({'tab': tab16, 'msk': msk, 'idx': np.ascontiguousarray(wr)})

    import time as _time
    t0 = _time.time()
    res = run_bass_kernel_spmd(nc, in_maps, core_ids=list(range(N_CORES)))
    LAST_STATS['exec_wall_s'] = round(_time.time() - t0, 3)
    if res.exec_time_ns:
        LAST_STATS['hw_exec_ns'] = res.exec_time_ns

    planars = []
    for core in range(N_CORES):
        od = np.asarray(res.results[core]['out'])    # [NGEN, NSUB, NGRP, 512]
        vals = od.transpose(0, 2, 1, 3).reshape(NGEN, NGRP, R, 4)
        planar = vals[:, :, UROW, :].reshape(NGEN * GPIX, 4)[:PIX]
        planars.append(planar)
    return planars


# revision 18
# speedup vs baseline: 1.0306x; 1.0014x over previous
import os
import sys
sys.path.insert(0, '/opt/trn_rl_repo')
import numpy as np

try:
    import concourse.bass as bass
    from concourse import bacc, tile, mybir
    from concourse.bass_utils import run_bass_kernel_spmd
    _HAVE_BASS = True
except Exception:
    _HAVE_BASS = False

# ---- problem constants (hardcoded; kernel.py must be self-contained) ----
B, C, H, W = 4, 1, 1024, 1024
L = 16
UP = 2
N_CORES = 8
NPASS = 16                      # 4 ktypes x 4 rotations
ROWS = H // 2                   # shard: (batch, half) -> 8 shards of 512 rows
PIX = ROWS * W                  # 524288 pixels per core

# flat indirect-DMA gather geometry (one instruction per SBUF partition line)
# first row of every 32-offset burst and of every SWDGE packet (R/16 rows)
# receives a corrupted offset -> throwaway slots, discarded at unpack
R = 3840                        # rows per line: 30*128; packet stride 240
UROW = ((np.arange(R) % 32) != 0) & ((np.arange(R) % (R // 16)) != 0)
U = int(UROW.sum())             # 3712 useful pixels per line
NGRP = 8                        # pixel groups per generation (16 passes each)
GPIX = NGRP * U                 # 30752 pixels per generation
NGEN = -(-PIX // GPIX)          # 18 generations
NSUB = R * 4 // 512             # 31 matmul sub-chunks per generation

OFFSETS = {
    'h': ((0, 0), (0, 1), (0, 2), (0, 3)),
    'd': ((0, 0), (1, 1), (2, 2), (3, 3)),
    't': ((0, 0), (2, 1), (3, 1), (3, 2)),
    'b': ((0, 0), (1, 2), (1, 3), (2, 3)),
}
KTYPES = ('h', 'd', 't', 'b')

_nc_cache = {}
LAST_STATS = {}


def _build_nc():
    """Flat-gather kernel: per generation, 128 indirect-DMA gathers (one per
    partition line; line = pixel-group x pass) from a concatenated 16-pass
    table; PE mask-matmul sums the 16 pass-partitions of each group into
    PSUM; ACT copies to SBUF; HWDGE writes out."""
    if 'nc' in _nc_cache:
        return _nc_cache['nc']
    f32 = mybir.dt.float32
    nc = bacc.Bacc('TRN2', target_bir_lowering=False,
                   dynamic_dma_scratch_size=2 ** 16)
    tab_d = nc.dram_tensor('tab', [NPASS * L ** 4, 4], f32, kind='ExternalInput')
    msk_d = nc.dram_tensor('msk', [128, NGRP], f32, kind='ExternalInput')
    idx_d = nc.dram_tensor('idx', [NGEN, 128, R], mybir.dt.int32,
                           kind='ExternalInput')
    out_d = nc.dram_tensor('out', [NGEN, NSUB, NGRP, 512], f32,
                           kind='ExternalOutput')

    with tile.TileContext(nc) as tc:
        with tc.tile_pool(name='const', bufs=1) as cpool, \
             tc.tile_pool(name='idx', bufs=2) as ipool, \
             tc.tile_pool(name='acc', bufs=2) as apool, \
             tc.tile_pool(name='oc', bufs=4) as opool, \
             tc.tile_pool(name='psum', bufs=8, space='PSUM') as ppool:
            msk = cpool.tile([128, NGRP], f32, tag='msk')
            nc.sync.dma_start(msk[:], msk_d[:])
            for gen in range(NGEN):
                it = ipool.tile([128, R], mybir.dt.int32, tag='it')
                nc.sync.dma_start(it[:], idx_d[gen])
                acc = apool.tile([128, R, 4], f32, tag='acc')
                # last generation: only the line-groups that cover PIX; the
                # skipped partitions hold stale (finite) data from an earlier
                # generation and their output positions are discarded anyway
                ngrp_gen = -(-(PIX - gen * GPIX) // U) if gen == NGEN - 1 else NGRP
                for q in range(16 * ngrp_gen):
                    # the final (partial) group needs <=896 pixels; its first
                    # 1024 rows hold >=990 useful slots, so gather only those
                    short = (gen == NGEN - 1 and q >= 16 * (ngrp_gen - 1))
                    rq = 1024 if short else R
                    nc.gpsimd.indirect_dma_start(
                        out=acc[q:q + 1, 0:rq, :], out_offset=None, in_=tab_d[:],
                        in_offset=bass.IndirectOffsetOnAxis(
                            ap=it[:, q * (R // 128):q * (R // 128) + rq // 128],
                            axis=0),
                        compute_op=mybir.AluOpType.bypass)
                for s in range(NSUB):
                    ps = ppool.tile([NGRP, 512], f32, tag='ps')
                    nc.tensor.matmul(ps[:], msk[:],
                                     acc[:, s * 128:(s + 1) * 128, :],
                                     start=True, stop=True)
                    oc = opool.tile([NGRP, 512], f32, tag='oc')
                    nc.scalar.copy(oc[:], ps[:])
                    nc.sync.dma_start(out_d[gen, s], oc[:])
    nc.compile()
    _nc_cache['nc'] = nc
    return nc


def _rot_offsets(offs, r):
    out = []
    for (dy, dx) in offs:
        if r == 0:
            out.append((dy, dx))
        elif r == 1:
            out.append((dx, -dy))
        elif r == 2:
            out.append((-dy, -dx))
        else:
            out.append((-dx, dy))
    return out


def _chan_perm(r):
    """perm[p*2+q] = source channel (i*2+j) that lands on subpixel (p, q)."""
    perm = [0] * 4
    for i in range(2):
        for j in range(2):
            if r == 0:
                p, q = i, j
            elif r == 1:
                p, q = j, 1 - i
            elif r == 2:
                p, q = 1 - i, 1 - j
            else:
                p, q = 1 - j, i
            perm[p * 2 + q] = i * 2 + j
    return perm


def _pass_tables_and_indices(img, weights):
    pad = np.pad(img[:, 0].astype(np.uint16), ((0, 0), (3, 3), (3, 3)),
                 mode='reflect')
    tabs = np.empty((NPASS, L ** 4, 4), np.float32)
    idx_full = np.empty((NPASS, B, H, W), np.int32)
    pi = 0
    for kt in KTYPES:
        for r in range(4):
            perm = _chan_perm(r)
            tabs[pi] = weights[kt][:, perm].astype(np.float32) * 0.25
            taps = _rot_offsets(OFFSETS[kt], r)
            acc = np.zeros((B, H, W), np.uint16)
            for (dy, dx) in taps:
                acc = (acc << 4) + pad[:, 3 + dy:3 + dy + H, 3 + dx:3 + dx + W]
            idx_full[pi] = acc
            pi += 1
    return tabs, idx_full


# static line geometry, shared by packing and unpacking
_JMAP = np.where(UROW, np.cumsum(UROW) - 1, 0).astype(np.int64)   # [R]


def kernel(img_lr, h_weight, d_weight, t_weight, b_weight, L=16, upscale=2):
    img = np.asarray(img_lr)
    weights = {'h': np.asarray(h_weight), 'd': np.asarray(d_weight),
               't': np.asarray(t_weight), 'b': np.asarray(b_weight)}
    tabs, idx_full = _pass_tables_and_indices(img, weights)

    use_device = _HAVE_BASS and bool(int(os.environ.get('HDTBLUT_DEVICE', '1')))
    planars = None
    LAST_STATS.clear()
    if use_device:
        try:
            planars = _run_device(tabs, idx_full)
            LAST_STATS['device_ok'] = True
        except Exception:
            if int(os.environ.get('HDTBLUT_STRICT', '0')):
                raise
            LAST_STATS['device_ok'] = False
            planars = None
    if planars is None:
        planars = []
        for core in range(N_CORES):
            b_, half = core // 2, core % 2
            sl = idx_full[:, b_, half * ROWS:(half + 1) * ROWS]
            acc = np.zeros((PIX, 4), np.float32)
            for p in range(NPASS):
                acc += tabs[p][sl[p].reshape(-1)]
            planars.append(acc)

    out = np.empty((B, 1, H * UP, W * UP), np.float32)
    for core in range(N_CORES):
        b_, half = core // 2, core % 2
        planar = np.asarray(planars[core]).reshape(ROWS, W, 2, 2)
        blk = planar.transpose(0, 2, 1, 3).reshape(ROWS * 2, W * 2)
        out[b_, 0, half * ROWS * 2:(half + 1) * ROWS * 2] = blk
    return out


def _run_device(tabs, idx_full):
    nc = _build_nc()
    tab16 = tabs.reshape(NPASS * L ** 4, 4)
    msk = np.zeros((128, NGRP), np.float32)
    for q in range(128):
        msk[q, q // NPASS] = 1.0

    # static pixel map [NGEN, NGRP, R]
    base = (np.arange(NGEN)[:, None, None] * GPIX
            + np.arange(NGRP)[None, :, None] * U + _JMAP[None, None, :])
    base = np.minimum(base, PIX + GPIX - 1)

    in_maps = []
    for core in range(N_CORES):
        b_, half = core // 2, core % 2
        sl = idx_full[:, b_, half * ROWS:(half + 1) * ROWS]
        idx16 = (sl.reshape(NPASS, PIX)
                 + (np.arange(NPASS, dtype=np.int32) * L ** 4)[:, None])
        padded = np.concatenate(
            [idx16, np.zeros((NPASS, GPIX), np.int32)], axis=1)
        # offsets[gen, q=16g+p, j] = padded[p, base[gen, g, j]]
        offs = padded[:, base]                       # [NPASS, NGEN, NGRP, R]
        offs = offs.transpose(1, 2, 0, 3)            # [NGEN, NGRP, NPASS, R]
        offs = offs.reshape(NGEN, 128, R)            # line q = 16g+p
        # DGE consumption: o_j = it[j%128, j//128] within each line's 31-col view
        wr = offs.reshape(NGEN, 128, R // 128, 128).transpose(0, 3, 1, 2)
        wr = wr.reshape(NGEN, 128, R).astype(np.int32)
        in_maps.append({'tab': tab16, 'msk': msk, 'idx': np.ascontiguousarray(wr)})

    import time as _time
    t0 = _time.time()
    res = run_bass_kernel_spmd(nc, in_maps, core_ids=list(range(N_CORES)))
    LAST_STATS['exec_wall_s'] = round(_time.time() - t0, 3)
    if res.exec_time_ns:
        LAST_STATS['hw_exec_ns'] = res.exec_time_ns

    planars = []
    for core in range(N_CORES):
        od = np.asarray(res.results[core]['out'])    # [NGEN, NSUB, NGRP, 512]
        vals = od.transpose(0, 2, 1, 3).reshape(NGEN, NGRP, R, 4)
        planar = vals[:, :, UROW, :].reshape(NGEN * GPIX, 4)[:PIX]
        planars.append(planar)
    return planars
